# revision 48
# baseline (speedup 1.0000x reference)
"""Trainium2 Bass kernel for MultiHeadLegendreGraphConvLayer.

Math (per batch b):
    A_hat = adj + I                                   [N, N]
    d = rowsum(A_hat) ** -0.5                         [N]
    L = d[:, None] * A_hat * d[None, :]               [N, N]
    P_k = Legendre_k(L) elementwise, k = 0..4
    prop_k = P_k @ x                                  [N, F]
    hout = concat_k(prop_k) @ W2.T + b  (per-head linear, k-major features)
    y = hout @ w_out.T + b_out                        [N, 256]

Key restructuring:
  * Legendre polys are ELEMENTWISE in L and matmul is linear, so
    prop_k = sum_j C[k, j] * (L^{o j} @ x); the coefficient matrix C folds
    into the head weights on the host (wjt).  j = 0 monomial (ones @ x)
    folds into a bias beta computed on device from s = colsum(x).
  * adj is transposed ON THE HOST (adjT) so the contraction dim of the
    stage-1 matmuls is the partition dim with NO on-device transposes.
  * v = adjT_tile * d_n (column scale, one DVE op per tile) makes the
    elementwise powers v^j carry d_n^j automatically; the row scale d_m^j
    is folded into the four stationaries xt[j] = d^j * x.  So
    mj[j] = sum_m xt[j]^T @ v^j  is the fully-scaled (L^{o j} @ x)^T and
    needs only a PSUM->SBUF cast before stage 2.
  * d comes from a second, original-orientation copy of adj (adjO)
    streamed once through a DVE rowsum; dbc1 (the broadcast d_n row used
    by v) is built in 512-column chunks so n-block 0 of the main loop
    starts after only 4 row blocks of phase A.
  * adjT tiles live in QUAD tiles [128, 4, 2048]: the power ops u2/u3/u4
    run on [128, 4*512] slabs (one op per 4 m-tiles) to amortize the
    per-op engine init cost.  v stays per-tile (it needs the dbc1 slice).
  * Stage 3 computes y^T (of on partitions): the b_out+beta bias is then
    per-partition and fuses into the ACT PSUM->SBUF copy; the host
    transposes y back.

Sharding: data-parallel over batch B=8 across the 8 cores (one batch each);
all weights replicated.
"""

import numpy as np
import ml_dtypes

import concourse.bass as bass
import concourse.bacc as bacc
import concourse.tile as tile
import concourse.mybir as mybir
from concourse.bass_utils import run_bass_kernel_spmd

F32 = mybir.dt.float32
BF16 = mybir.dt.bfloat16
AF = mybir.ActivationFunctionType
OP = mybir.AluOpType

N = 2048
F = 128
OUT_F = 256
K = 4
NB = 4          # n-blocks of 512 columns
NW = 512        # n-block width
MT = 16         # m-tiles of 128
RB = 16         # row blocks of 128
NQ = 4          # quads of 4 m-tiles
P = 128

BF = ml_dtypes.bfloat16


def build_nc(reps=1, cfg=None):
    cfg = {**dict(u2="act", u3="dve", u4="dve/act", stbufs=4, upbufs=4,
                  mjsbufs=8, yobufs=2), **(cfg or {})}
    nc = bacc.Bacc("TRN2", target_bir_lowering=False, debug=False, num_devices=8)

    adjT = nc.dram_tensor("adjT", [N, N], BF16, kind="ExternalInput").ap()
    adjO = nc.dram_tensor("adjO", [N, N], BF16, kind="ExternalInput").ap()
    xb = nc.dram_tensor("xb", [P, N], BF16, kind="ExternalInput").ap()
    wb = nc.dram_tensor("wb", [P, 7 * OUT_F], BF16, kind="ExternalInput").ap()
    bhbo = nc.dram_tensor("bhbo", [P, 4], F32, kind="ExternalInput").ap()
    yT = nc.dram_tensor("yT", [OUT_F, N], F32, kind="ExternalOutput").ap()

    def _sel(spec, i):
        opts = spec.split("/")
        return opts[i % len(opts)]

    def _mul(sel, out_, a_, b_):
        if sel == "dve":
            nc.vector.tensor_mul(out_, a_, b_)
        elif sel == "pool":
            nc.gpsimd.tensor_mul(out_, a_, b_)
        elif sel == "act":
            nc.scalar.activation(out_, a_, AF.Square)
        else:
            raise ValueError(sel)

    with tile.TileContext(nc) as tc:
        with (
            tc.tile_pool(name="singles", bufs=1) as singles,
            tc.tile_pool(name="stage", bufs=cfg["stbufs"]) as stage,
            tc.tile_pool(name="upool", bufs=cfg["upbufs"]) as upool,
            tc.tile_pool(name="mjs", bufs=cfg["mjsbufs"]) as mjsp,
            tc.tile_pool(name="yout", bufs=cfg["yobufs"]) as youtp,
            tc.tile_pool(name="mj_ps", bufs=1, space="PSUM") as mj_ps,
            tc.tile_pool(name="yp_ps", bufs=4, space="PSUM") as yp_ps,
            tc.tile_pool(name="drampool", bufs=2, space="DRAM") as drampool,
        ):
          for _rep in range(reps):
            d_dram = drampool.tile([N], F32, tag="d_dram", name="d_dram")
            # ---- persistent SBUF tensors -------------------------------
            x_all = singles.tile([P, N], BF16, tag="x_all")        # [m_p, t*128+f]
            xt = [singles.tile([P, N], BF16, tag=f"xt{j}", name=f"xt{j}")
                  for j in range(4)]
            aT = [singles.tile([P, NQ, N], BF16, tag=f"aT{q}", name=f"aT{q}")
                  for q in range(NQ)]
            wb_sb = singles.tile([P, 7 * OUT_F], BF16, tag="wb")
            wcT_sb = wb_sb[:, 0:4 * OUT_F]
            w0t_sb = wb_sb[:, 4 * OUT_F:5 * OUT_F]
            woutT_sb = wb_sb[:, 5 * OUT_F:7 * OUT_F]
            bhbo_sb = singles.tile([P, 4], F32, tag="bhbo")
            bh_sb = bhbo_sb[:, 0:2]
            bo_sb = bhbo_sb[:, 2:4]
            ones_sb = singles.tile([P, 1], BF16, tag="ones")
            rs_all = singles.tile([P, RB], F32, tag="rs")
            d_pt = singles.tile([P, RB], F32, tag="d1")
            dp2 = singles.tile([P, RB], F32, tag="d2")
            dp3 = singles.tile([P, RB], F32, tag="d3")
            dp4 = singles.tile([P, RB], F32, tag="d4")
            dbc1 = singles.tile([P, N], BF16, tag="dbc1")
            s_f = singles.tile([P, 1], F32, tag="s_f")
            s_bf = singles.tile([P, 1], BF16, tag="s_bf")
            t1_f = singles.tile([P, 2], F32, tag="t1f")
            t1_bf = singles.tile([P, 2], BF16, tag="t1bf")
            beta_f = singles.tile([P, 2], F32, tag="betaf")

            ao_tiles = {}

            def emit_adjO_dma(g):
                # [128, 2, 2048] view of 2 consecutive 128-row blocks
                src = bass.AP(
                    tensor=adjO.tensor, offset=adjO.offset + g * 2 * P * N,
                    ap=[[N, P], [P * N, 2], [1, N]],
                )
                ao2 = stage.tile([P, 2, N], BF16, tag="ao", name="ao2")
                nc.sync.dma_start(out=ao2[:], in_=src)
                ao_tiles[g] = ao2

            def emit_aT_dma(q, nb):
                # column chunk [4 m-tiles, 512 cols] of adjT quad q
                src = bass.AP(
                    tensor=adjT.tensor,
                    offset=adjT.offset + q * 4 * P * N + nb * NW,
                    ap=[[N, P], [P * N, 4], [1, NW]],
                )
                nc.scalar.dma_start(
                    out=aT[q][:, :, nb * NW:(nb + 1) * NW], in_=src
                )

            # ---- DMA priority order: adjO half-groups (gate d) and the
            # nb=0 adjT column chunks interleaved so each quad's d and
            # data land just in time; the stage pool's WAR deps pace the
            # adjO stream against rowsum consumption.
            emit_adjO_dma(0)
            nc.scalar.dma_start(out=x_all[:], in_=xb)
            emit_adjO_dma(1)
            emit_aT_dma(0, 0)
            emit_adjO_dma(2)
            emit_adjO_dma(3)
            emit_aT_dma(1, 0)
            emit_adjO_dma(4)
            emit_adjO_dma(5)
            emit_aT_dma(2, 0)
            emit_adjO_dma(6)
            emit_adjO_dma(7)
            emit_aT_dma(3, 0)
            nc.scalar.dma_start(out=wb_sb[:], in_=wb)
            nc.scalar.dma_start(out=bhbo_sb[:], in_=bhbo)
            nc.vector.memset(ones_sb[:], 1.0)

            # ---- phase A: rowsums -> d, d^j; build dbc1 in 512-col
            # chunks so nb=0 can start after group 0.  xt[j] scaling is
            # deferred into the first n-block's quad loop. ----------------
            def emit_phaseA_group(gb):
                """Rowsums + d chain for rows [2gb*128, (2gb+2)*128), plus
                the dbc chunk after each odd half-group.  Interleaved with
                nb=0 quads so the in-order DVE/ACT queues pipeline phase A
                with stage 1."""
                ao2 = ao_tiles.pop(gb)
                for b in range(2):
                    r = gb * 2 + b
                    nc.vector.tensor_scalar(
                        ao2[:, b, :], ao2[:, b, :], 1.0, 0.0, OP.mult, OP.add,
                        accum_out=rs_all[:, r:r + 1],
                    )
                # d = (rowsum + 1) ** -0.5 and powers, batched per group
                gsl = slice(gb * 2, gb * 2 + 2)
                nc.vector.tensor_scalar_add(dp2[:, gsl], rs_all[:, gsl], 1.0)
                nc.scalar.sqrt(dp2[:, gsl], dp2[:, gsl])
                nc.vector.reciprocal(d_pt[:, gsl], dp2[:, gsl])
                nc.vector.tensor_mul(dp2[:, gsl], d_pt[:, gsl], d_pt[:, gsl])
                nc.vector.tensor_mul(dp3[:, gsl], dp2[:, gsl], d_pt[:, gsl])
                nc.vector.tensor_mul(dp4[:, gsl], dp2[:, gsl], dp2[:, gsl])
                # d_dram[2g*256 + b*128 + p] = d_pt[p, 2g+b] -- SWDGE
                # (gpsimd): tiny transfers, skips the busy HWDGE FIFOs.
                # Only needed by the mjs evac (~25us in), not by stage 1.
                d_dst = bass.AP(
                    tensor=d_dram.tensor, offset=d_dram.offset + gb * 2 * P,
                    ap=[[1, P], [P, 2]],
                )
                nc.gpsimd.dma_start(out=d_dst, in_=d_pt[:, gsl])
                if gb % 2 == 1:
                    # dbc1[:, nb chunk] = d[n] broadcast across partitions,
                    # cast f32 -> bf16 inline (SWDGE cast-DMA)
                    nbv = gb // 2
                    nsl = slice(nbv * NW, (nbv + 1) * NW)
                    dbc_src = bass.AP(
                        tensor=d_dram.tensor, offset=d_dram.offset + nbv * NW,
                        ap=[[0, P], [1, NW]],
                    )
                    nc.gpsimd.dma_start(out=dbc1[:, nsl], in_=dbc_src)

            def emit_beta():
                # s = colsum(x); beta = w_out @ (W0 @ s + b_h) + b_out
                s_ps = yp_ps.tile([P, 2], F32, tag="yp", name="s_ps")
                for t in range(MT):
                    nc.tensor.matmul(
                        s_ps[:, 0:1], x_all[:, t * P:(t + 1) * P], ones_sb[:],
                        start=(t == 0), stop=(t == MT - 1),
                    )
                nc.scalar.copy(s_f[:], s_ps[:, 0:1])
                nc.vector.tensor_copy(s_bf[:], s_f[:])
                t1_ps = yp_ps.tile([P, 2], F32, tag="yp", name="t1_ps")
                for h in range(2):
                    nc.tensor.matmul(
                        t1_ps[:, h:h + 1], w0t_sb[:, h * P:(h + 1) * P], s_bf[:],
                        start=True, stop=True,
                    )
                nc.scalar.copy(t1_f[:], t1_ps[:])
                nc.vector.tensor_add(t1_f[:], t1_f[:], bh_sb[:])
                nc.vector.tensor_copy(t1_bf[:], t1_f[:])
                beta_ps = yp_ps.tile([P, 2], F32, tag="yp", name="beta_ps")
                for of_h in range(2):
                    for h in range(2):
                        nc.tensor.matmul(
                            beta_ps[:, of_h:of_h + 1],
                            woutT_sb[:, h * OUT_F + of_h * P: h * OUT_F + (of_h + 1) * P],
                            t1_bf[:, h:h + 1],
                            start=(h == 0), stop=(h == 1),
                        )
                nc.scalar.copy(beta_f[:], beta_ps[:])
                nc.vector.tensor_add(beta_f[:], beta_f[:], bo_sb[:])

            # ---- main loop (beta emitted after nb=0 stage 1 so the PE
            # queue reaches stage-1 matmuls without waiting on weights) ---
            for nb in range(NB):
                nsl = slice(nb * NW, (nb + 1) * NW)
                mj = [mj_ps.tile([P, NW], F32, tag=f"mj{j}", name=f"mj{j}")
                      for j in range(4)]
                for q in range(NQ):
                    if nb == 0:
                        emit_phaseA_group(2 * q)
                        emit_phaseA_group(2 * q + 1)
                        # deferred xt[j] row scaling for this quad's m-tiles
                        for i in range(NQ):
                            r = q * NQ + i
                            rsl = slice(r, r + 1)
                            xsl = slice(r * P, (r + 1) * P)
                            for j, dp in enumerate([d_pt, dp2, dp3, dp4]):
                                nc.vector.tensor_scalar(
                                    xt[j][:, xsl], x_all[:, xsl], dp[:, rsl],
                                    None, OP.mult,
                                )
                    # raw powers of the adjT quad slab; d_n^j applies at the
                    # mjs evac, d_m^j is folded into the xt stationaries
                    uq = aT[q][:, :, nsl]
                    u2q = upool.tile([P, NQ, NW], BF16, tag="u2")
                    _mul(_sel(cfg["u2"], q), u2q[:], uq, uq)
                    u3q = upool.tile([P, NQ, NW], BF16, tag="u3")
                    if _sel(cfg["u3"], q) == "act":
                        raise ValueError("u3 needs tensor_tensor")
                    _mul(_sel(cfg["u3"], q), u3q[:], u2q[:], uq)
                    u4q = upool.tile([P, NQ, NW], BF16, tag="u4")
                    _mul(_sel(cfg["u4"], q), u4q[:], u2q[:], u2q[:])
                    for i in range(NQ):
                        m = q * NQ + i
                        msl = slice(m * P, (m + 1) * P)
                        for j, rt in enumerate(
                            [aT[q][:, i, nsl], u2q[:, i, :],
                             u3q[:, i, :], u4q[:, i, :]]
                        ):
                            nc.tensor.matmul(
                                mj[j][:], xt[j][:, msl], rt,
                                start=(m == 0), stop=(m == MT - 1),
                            )
                    if nb < NB - 1:
                        # prefetch this quad's next column chunk
                        emit_aT_dma(q, nb + 1)
                if nb == 0:
                    emit_beta()
                # dbc_j[:, n] = d_n^j for this n-block (from dbc1 chunk)
                dbc2 = upool.tile([P, NW], BF16, tag="dbc2")
                nc.scalar.activation(dbc2[:], dbc1[:, nsl], AF.Square)
                dbc3 = upool.tile([P, NW], BF16, tag="dbc3")
                nc.vector.tensor_mul(dbc3[:], dbc2[:], dbc1[:, nsl])
                dbc4 = upool.tile([P, NW], BF16, tag="dbc4")
                nc.scalar.activation(dbc4[:], dbc2[:], AF.Square)
                # evac mj to bf16 with the d_n^j column scale
                mjs = []
                for j, dbcj in enumerate([dbc1[:, nsl], dbc2[:], dbc3[:], dbc4[:]]):
                    t = mjsp.tile([P, NW], BF16, tag="mjs", name="mjs_t")
                    nc.vector.tensor_tensor(t[:], mj[j][:], dbcj, OP.mult)
                    mjs.append(t)
                # fused stage 2+3 (w_out folded into the per-j weights on
                # the host): y^T[of, n] = sum_j Wc_j mjs_j + beta
                yt = youtp.tile([P, 2, NW], F32, tag="yt")
                for c in range(2):
                    yp = yp_ps.tile([P, NW], F32, tag="yp", name="yp")
                    for j in range(4):
                        nc.tensor.matmul(
                            yp[:], wcT_sb[:, j * OUT_F + c * P: j * OUT_F + (c + 1) * P],
                            mjs[j][:], start=(j == 0), stop=(j == 3),
                        )
                    nc.vector.tensor_scalar(
                        yt[:, c, :], yp[:], beta_f[:, c:c + 1], None, OP.add
                    )
                y_dst = bass.AP(
                    tensor=yT.tensor, offset=yT.offset + nb * NW,
                    ap=[[N, P], [P * N, 2], [1, NW]],
                )
                nc.sync.dma_start(out=y_dst, in_=yt[:])

    nc.compile()
    return nc


def host_prep(w_heads, b_heads, w_out, b_out):
    """Fold Legendre coefficients + transposes into device weight layouts."""
    H, OH, CF = w_heads.shape
    W2 = np.asarray(w_heads, np.float64).reshape(H * OH, CF)   # [256, 640]
    C = np.zeros((5, 5))
    C[0, 0] = 1.0
    C[1, 1] = 1.0
    C[2, :3] = [-0.5, 0.0, 1.5]
    C[3, :4] = [0.0, -1.5, 0.0, 2.5]
    C[4, :5] = [0.375, 0.0, -3.75, 0.0, 4.375]
    Wj = []
    for j in range(5):
        acc = np.zeros((H * OH, F))
        for k in range(5):
            if C[k, j] != 0.0:
                acc += C[k, j] * W2[:, k * F:(k + 1) * F]
        Wj.append(acc)

    w0t = Wj[0].T.astype(np.float32)                           # [128, 256]
    wf = np.asarray(w_out, np.float64)
    # woutT[p, h*256+of] = w_out[of, h*128+p]   (beta path, of on free dim)
    woutT = wf.T.reshape(2, P, OUT_F).transpose(1, 0, 2).reshape(P, 2 * OUT_F)
    # Fused stage 2+3 weights: Wc_j = w_out @ Wj^T  [256 of, 128 f];
    # wcT[p=f, j*256 + c*128 + of'] = Wc_j[c*128+of', p]
    wcT = np.zeros((P, 4 * OUT_F))
    for j in range(1, 5):
        Wc = wf @ Wj[j]                                        # [256, 128]
        for c in range(2):
            wcT[:, (j - 1) * OUT_F + c * P:(j - 1) * OUT_F + (c + 1) * P] = \
                Wc[c * P:(c + 1) * P, :].T
    bh = np.asarray(b_heads, np.float64).reshape(2, P).T.astype(np.float32)
    bo = np.asarray(b_out, np.float64).reshape(2, P).T.astype(np.float32)
    wblob = np.concatenate(
        [wcT, w0t, woutT.astype(np.float32)], axis=1
    )
    return {
        "wb": np.ascontiguousarray(wblob.astype(BF)),
        "bhbo": np.ascontiguousarray(np.concatenate([bh, bo], axis=1)),
    }


_NC_CACHE = {}


def _get_nc():
    if "nc" not in _NC_CACHE:
        _NC_CACHE["nc"] = build_nc()
    return _NC_CACHE["nc"]


def make_in_maps(x, adj, w_heads, b_heads, w_out, b_out):
    weights = host_prep(w_heads, b_heads, w_out, b_out)
    B = x.shape[0]
    in_maps = []
    for b in range(B):
        m = dict(weights)
        a16 = np.asarray(adj[b], np.float32).astype(BF)
        m["adjO"] = np.ascontiguousarray(a16)
        m["adjT"] = np.ascontiguousarray(a16.T)
        # xb[p, t*128+f] = x[t*128+p, f]  (m on partitions)
        xbT = (np.asarray(x[b], np.float32).astype(BF)
               .reshape(MT, P, F).transpose(1, 0, 2).reshape(P, MT * F))
        m["xb"] = np.ascontiguousarray(xbT)
        in_maps.append(m)
    return in_maps


def kernel(x, adj, w_heads, b_heads, w_out, b_out):
    x = np.asarray(x)
    adj = np.asarray(adj)
    in_maps = make_in_maps(x, adj, w_heads, b_heads, w_out, b_out)
    nc = _get_nc()
    res = run_bass_kernel_spmd(nc, in_maps, list(range(len(in_maps)))).results
    return np.stack(
        [np.ascontiguousarray(r["yT"].T) for r in res]
    ).astype(np.float32)
